# revision 6
# baseline (speedup 1.0000x reference)
"""Trainium2 Bass kernel: single-head causal attention, SPMD over 8 NeuronCores.

Problem: x [4, 2048, 1024] f32; Wq/Wk/Wv [1024, 64]; bq/bk/bv [64].
  q,k,v = x@W + b ; out = softmax(causal(q k^T / 8)) @ v  -> [4, 2048, 64]

Sharding (uniform SPMD structure on every core):
  core c -> batch b = c//2 ; query chunks (cA, cB) = (c%2, 3-c%2), 512 rows
  each (pairing an early with a late chunk balances causal work).  Every core
  computes K/V for its batch's full 2048 rows.

Key layout trick: the k-axis is permuted PER CORE to chunk order
  [cA, 1-cA, 5-cB, cB], so the core's own query columns sit at STATIC
  positions 0:512 and 1536:2048 of the K/V input -- Q projection needs no
  separate input tensor.  Causality comes from host-precomputed per-core 0/1
  mask tiles (diagonal tiles) and exp-bias kills (fully-masked tiles), which
  absorb the permutation.

  Projections produce Q^T/K^T/V^T [64, rows]; scores are computed transposed
  ([k_part, q_free]) so the attention-weight matrix feeds the AV matmul as
  the moving operand; V is re-transposed through 16 small PE transposes; a
  65th "ones" row on the V tiles makes the AV matmul accumulate the softmax
  denominator for free.  Score matmuls (K=64) are row-packed in pairs into
  disjoint PE row-groups; the partition-64:127 K^T/Q^T duplicates they need
  are produced by extra partition-shifted bias-adds straight from PSUM (DVE/
  ACT can read partitions 0:64 and write 64:128), NOT by SBUF->SBUF DMA --
  a dma_start costs ~0.65us sequencer issue + ~2us completion.

Schedule: the hard resource is the ACT engine (24 exp tiles x ~0.58us,
  1 elem/cycle/lane regardless of dtype) -- the emission order exists to
  start the exp chain ASAP and keep it gapless:
  - FEW LARGE input DMAs (one dma_start spans all 16 SDMA engines at
    ~340GB/s) split across the sync/scalar/gpsimd queues so issue (~0.65us
    each, serialized per queue) overlaps; xk streams smallest-first so the
    first proj matmul starts right after startup (~8us);
  - the h1 half of x is split so the slot-B query columns (shared with
    kv-sub1) land before kv-sub0, unblocking slot-B scores early;
  - slot-A attention interleaves with proj h1 on the PE (PSUM banks:
    proj 3 + score 3 + av 2 = 8; one 2-bank av pool is reused A->B);
  - slot-A tail (merge on DVE + transpose + store) is emitted AFTER all
    slot-B scores/projections so it never blocks the PE queue ahead of
    the exp spine; slot-B merge uses ACT (free after the last exp).

dtypes: fp16 SBUF operands, fp32 PSUM accumulation, fp32 normalize.
"""

import os
import sys

import numpy as np

if "/opt/trn_rl_repo" not in sys.path:
    sys.path.insert(0, "/opt/trn_rl_repo")

B, S, D, H = 4, 2048, 1024, 64
CH = 512          # query chunk width
QR = 2 * CH       # query rows per core
NKT = S // 128    # 16 k-tiles of 128
SCALE = 1.0 / np.sqrt(H)

# cst16 column layout (fp16): [0:1536] -> scalar DMA, [1536:] -> gpsimd DMA
C_WKV = 0          # [128, 1024] 8 kt blocks of [128, 128]
C_WQ = 1024        # [128, 512]  8 kt blocks of [128, 64]
C_IDV = 1536       # [128, 64]   eye(64) stacked twice
C_ID16 = 1600      # [65, 65]    eye(65)
C16_N = 1696       # padded total

# cst32 column layout (f32)
C_BKV = 0          # [128, 1]
C_BQ = 1           # [64, 1]
C_THRB = 2         # [128, 32] exp bias: 0 or -1e5 (fully-masked kill)
C32_N = 34

# xk chunk column ranges (smallest-first for early PE start)
XCH = [(0, 1024), (1024, 2048), (2048, 4096), (4096, 6144), (6144, 8192),
       (8192, 12288), (12288, 16384)]

_CACHE = {}


def _build_nc():
    import concourse.bacc as bacc
    import concourse.mybir as mybir
    import concourse.tile as tile

    DT = mybir.dt.float16
    F32 = mybir.dt.float32
    Exp = mybir.ActivationFunctionType.Exp
    Copy = mybir.ActivationFunctionType.Copy
    Ident = mybir.ActivationFunctionType.Identity
    mult = mybir.AluOpType.mult
    add = mybir.AluOpType.add

    nc = bacc.Bacc("TRN2", target_bir_lowering=False, debug=False, num_devices=8)

    # xk (k-permuted x^T, partition-major), columns:
    #   [kt*1024 + s*512]        h0 tile kt, sub s   (k-positions 0:1024)
    #   [8192  + kt*512]         h1 tile kt, sub1 == slot-B q (pos 1536:2048)
    #   [12288 + kt*512]         h1 tile kt, sub0    (pos 1024:1536)
    xk = nc.dram_tensor("xk", [128, 16 * 1024], DT, kind="ExternalInput")
    cst16 = nc.dram_tensor("cst16", [128, C16_N], DT, kind="ExternalInput")
    cst32 = nc.dram_tensor("cst32", [128, C32_N], F32, kind="ExternalInput")
    mskd = nc.dram_tensor("mskd", [128, 8 * 512], DT, kind="ExternalInput")
    out = nc.dram_tensor("out", [128, 8 * H], DT, kind="ExternalOutput")

    with tile.TileContext(nc) as tc:
        with (
            tc.tile_pool(name="const", bufs=1) as cp,
            tc.tile_pool(name="work", bufs=8) as wp,
            tc.tile_pool(name="epi", bufs=4) as ep,
        ):
            # ---- head: large DMAs, issue spread over 3 queues ----
            cst16_sb = cp.tile([128, C16_N], DT, tag="cst16", name="cst16")
            nc.scalar.dma_start(cst16_sb[:, 0:1536], cst16[:, 0:1536])
            xc = []
            for ci, (c0, c1) in enumerate(XCH):
                t = cp.tile([128, c1 - c0], DT, tag=f"xc{ci}", name=f"xc{ci}")
                xc.append(t)
                nc.sync.dma_start(t[:], xk[:, c0:c1])
            cst32_sb = cp.tile([128, C32_N], F32, tag="cst32", name="cst32")
            nc.gpsimd.dma_start(cst32_sb[:], cst32[:])
            msk_sb = cp.tile([128, 8 * 512], DT, tag="msk", name="msk")
            nc.gpsimd.dma_start(msk_sb[:], mskd[:])
            nc.gpsimd.dma_start(cst16_sb[:, 1536:C16_N], cst16[:, 1536:C16_N])

            wkv_sb = cst16_sb[:, C_WKV:C_WKV + 1024]
            wq_sb = cst16_sb[:, C_WQ:C_WQ + 512]
            idv_sb = cst16_sb[:, C_IDV:C_IDV + H]
            id16_sb = cst16_sb[0:H + 1, C_ID16:C_ID16 + H + 1]
            bkv_sb = cst32_sb[:, C_BKV:C_BKV + 1]
            bq_sb = cst32_sb[0:H, C_BQ:C_BQ + 1]
            thrb_sb = cst32_sb[:, C_THRB:C_THRB + 2 * NKT]

            def x0(kt, s):      # h0 tile kt, 512-col sub s (s=0 is also qA)
                ci = [0, 1, 2, 2, 3, 3, 4, 4][kt]
                off = kt * 1024 + s * 512 - XCH[ci][0]
                return xc[ci][:, off:off + 512]

            def x1b(kt):        # h1 tile kt, sub1 (== slot-B q rows)
                return xc[5][:, kt * 512:(kt + 1) * 512]

            def x1a(kt):        # h1 tile kt, sub0
                return xc[6][:, kt * 512:(kt + 1) * 512]

            kvT_sb = cp.tile([128, S], DT, tag="kvT", name="kvT")  # 0:64 K^T, 64:128 V^T
            qT_sb = cp.tile([H, QR], DT, tag="qT", name="qT")      # A cols 0:512, B 512:1024
            v_sb = cp.tile([128, NKT * (H + 1)], DT, tag="v", name="v")
            # duplicates at partitions 64:127 for row-packed score pairs
            ktd_sb = cp.tile([128, S], DT, tag="ktd", name="ktd")
            qTd_sb = cp.tile([128, QR], DT, tag="qTd", name="qTd")
            vtd_sb = cp.tile([64, S], DT, tag="vtd", name="vtd")
            o_all = cp.tile([128, 8 * H], DT, tag="oall", name="oall")

            nc.vector.memset(v_sb[:], 1.0)

            # ---- phase h0 projections (3 PSUM banks) ----
            pp0 = tc.alloc_tile_pool(name="proj_ps0", bufs=1, space="PSUM")
            kv_ps = [pp0.tile([128, 512], F32, tag=f"kvps{s}",
                              name=f"kvps{s}") for s in range(2)]
            q_ps = pp0.tile([H, 512], F32, tag="qps", name="qps")
            for kt in range(8):
                nc.tensor.matmul(
                    kv_ps[0][:], wkv_sb[:, kt * 128:(kt + 1) * 128],
                    x0(kt, 0), start=(kt == 0), stop=(kt == 7))
                nc.tensor.matmul(
                    kv_ps[1][:], wkv_sb[:, kt * 128:(kt + 1) * 128],
                    x0(kt, 1), start=(kt == 0), stop=(kt == 7))
                nc.tensor.matmul(
                    q_ps[:], wq_sb[:, kt * H:(kt + 1) * H],
                    x0(kt, 0), start=(kt == 0), stop=(kt == 7))
            # epilogue h0: qT/qTd on ACT, kvT/ktd/vtd on DVE; the partition-
            # shifted dup adds read PSUM 0:64 and write SBUF 64:128 directly
            nc.scalar.activation(qT_sb[:, 0:512], q_ps[:], Ident,
                                 bias=bq_sb[:])
            nc.scalar.activation(qTd_sb[H:128, 0:512], q_ps[:], Ident,
                                 bias=bq_sb[:])
            for sub in range(2):
                cs = slice(sub * 512, (sub + 1) * 512)
                nc.vector.tensor_scalar(ktd_sb[H:128, cs], kv_ps[sub][0:H, :],
                                        bkv_sb[0:H, :], None, add)
                nc.vector.tensor_scalar(kvT_sb[:, cs], kv_ps[sub][:],
                                        bkv_sb[:], None, add)
            for sub in range(2):
                cs = slice(sub * 512, (sub + 1) * 512)
                nc.vector.tensor_scalar(vtd_sb[:, cs], kv_ps[sub][H:128, :],
                                        bkv_sb[H:128, :], None, add)
            pp0.release()

            sp = tc.alloc_tile_pool(name="score_ps", bufs=3, space="PSUM")
            avp = tc.alloc_tile_pool(name="av_ps", bufs=1, space="PSUM")
            pp1 = tc.alloc_tile_pool(name="proj_ps1", bufs=1, space="PSUM")

            def v_transpose(pr):
                k0, k1 = 2 * pr, 2 * pr + 1
                t0 = sp.tile([128, H], DT, tag="score", name="vtr0")
                nc.tensor.transpose(
                    t0[:], vtd_sb[:, k0 * 128:(k0 + 1) * 128],
                    idv_sb[0:H, :], tile_position=(0, 0))
                t1 = sp.tile([128, H], DT, tag="score", name="vtr1")
                nc.tensor.transpose(
                    t1[:], kvT_sb[64:128, k1 * 128:(k1 + 1) * 128],
                    idv_sb[64:64 + H, :], tile_position=(64, 0))
                nc.vector.tensor_copy(
                    v_sb[:, k0 * (H + 1):k0 * (H + 1) + H], t0[:])
                nc.vector.tensor_copy(
                    v_sb[:, k1 * (H + 1):k1 * (H + 1) + H], t1[:])

            def score_pair(slot, kt0, kt1):
                s0 = sp.tile([128, 512], F32, tag="score", name="score0")
                nc.tensor.matmul(
                    s0[:], kvT_sb[0:H, kt0 * 128:(kt0 + 1) * 128],
                    qT_sb[:, slot * 512:(slot + 1) * 512],
                    start=True, stop=True, tile_position=(0, 0))
                s1 = sp.tile([128, 512], F32, tag="score", name="score1")
                nc.tensor.matmul(
                    s1[:], ktd_sb[H:128, kt1 * 128:(kt1 + 1) * 128],
                    qTd_sb[H:128, slot * 512:(slot + 1) * 512],
                    start=True, stop=True, tile_position=(64, 0))
                return s0, s1

            def exp_pair(slot, kt0, kt1, s0, s1):
                """exp on ACT; diag tiles get the host-built ge mask
                multiplied in on DVE."""
                res = []
                for kt, s_ps in zip((kt0, kt1), (s0, s1)):
                    idx = slot * NKT + kt
                    w_sb = wp.tile([128, 512], DT, tag="wexp", name="wexp")
                    nc.scalar.activation(w_sb[:], s_ps[:], Exp,
                                         bias=thrb_sb[:, idx:idx + 1],
                                         scale=float(SCALE))
                    diag = (slot == 0 and kt < 4) or (slot == 1 and kt >= 12)
                    if diag:
                        m = kt if slot == 0 else kt - 8
                        wm_sb = wp.tile([128, 512], DT, tag="wm", name="wm")
                        nc.vector.tensor_tensor(
                            wm_sb[:], w_sb[:],
                            msk_sb[:, m * 512:(m + 1) * 512], mult)
                        res.append(wm_sb)
                    else:
                        res.append(w_sb)
                return res

            def av_accum(av_e, av_o, kt, w_av, first, last):
                vs = slice(kt * (H + 1), (kt + 1) * (H + 1))
                nc.tensor.matmul(
                    av_e[:], v_sb[0:H, vs], w_av[0:H, :],
                    start=first, stop=last, tile_position=(0, 0))
                nc.tensor.matmul(
                    av_o[:], v_sb[H:128, vs], w_av[H:128, :],
                    start=first, stop=last, tile_position=(64, 0))

            # ---- slot A scores (kvT h0 ready) + V transposes 0..3 ----
            avA_e = avp.tile([H + 1, 512], F32, tag="avE", name="avE")
            avA_o = avp.tile([H + 1, 512], F32, tag="avO", name="avO")
            sA = [score_pair(0, 2 * p, 2 * p + 1) for p in range(2)]
            wA = [exp_pair(0, 2 * p, 2 * p + 1, *sA[p]) for p in range(2)]
            for pr in range(2):
                v_transpose(pr)
            sA2 = [score_pair(0, 4 + 2 * p, 5 + 2 * p) for p in range(2)]
            wA2 = [exp_pair(0, 4 + 2 * p, 5 + 2 * p, *sA2[p]) for p in range(2)]
            for pr in range(2, 4):
                v_transpose(pr)

            # ---- phase h1 group 1: kv sub1 + q (chunk 5) ----
            kv1_ps = pp1.tile([128, 512], F32, tag="kvps1b", name="kvps1b")
            q1_ps = pp1.tile([H, 512], F32, tag="qps1", name="qps1")
            for kt in range(8):
                nc.tensor.matmul(
                    kv1_ps[:], wkv_sb[:, kt * 128:(kt + 1) * 128],
                    x1b(kt), start=(kt == 0), stop=(kt == 7))
                nc.tensor.matmul(
                    q1_ps[:], wq_sb[:, kt * H:(kt + 1) * H],
                    x1b(kt), start=(kt == 0), stop=(kt == 7))
            # epilogue: qT/qTd B first (unblock slot-B scores kt0..7)
            nc.vector.tensor_scalar(qT_sb[:, 512:1024], q1_ps[:],
                                    bq_sb[:], None, add)
            nc.vector.tensor_scalar(qTd_sb[H:128, 512:1024], q1_ps[:],
                                    bq_sb[:], None, add)
            nc.vector.tensor_scalar(ktd_sb[H:128, 3 * 512:4 * 512],
                                    kv1_ps[0:H, :], bkv_sb[0:H, :], None, add)
            nc.vector.tensor_scalar(kvT_sb[:, 3 * 512:4 * 512], kv1_ps[:],
                                    bkv_sb[:], None, add)
            nc.vector.tensor_scalar(vtd_sb[:, 3 * 512:4 * 512],
                                    kv1_ps[H:128, :], bkv_sb[H:128, :],
                                    None, add)

            # ---- slot B scores kt 0..7 (feed ACT right after slot A exps)
            kts = list(range(8)) + [12, 13, 14, 15, 8, 9, 10, 11]
            wB = {}
            for p in range(4):
                kt0, kt1 = kts[2 * p], kts[2 * p + 1]
                s0, s1 = score_pair(1, kt0, kt1)
                wB[p] = exp_pair(1, kt0, kt1, s0, s1)

            # slot A AV p0,p1 (exps A p0,p1 done early)
            for p in range(2):
                for j in range(2):
                    av_accum(avA_e, avA_o, 2 * p + j, wA[p][j],
                             2 * p + j == 0, False)

            # slot B kt 12..15 V transposes + scores
            for pr in (6, 7):
                v_transpose(pr)
            for p in (4, 5):
                kt0, kt1 = kts[2 * p], kts[2 * p + 1]
                s0, s1 = score_pair(1, kt0, kt1)
                wB[p] = exp_pair(1, kt0, kt1, s0, s1)

            # ---- phase h1 group 2: kv sub0 (chunk 6) ----
            kv0_ps = pp1.tile([128, 512], F32, tag="kvps1a", name="kvps1a")
            for kt in range(8):
                nc.tensor.matmul(
                    kv0_ps[:], wkv_sb[:, kt * 128:(kt + 1) * 128],
                    x1a(kt), start=(kt == 0), stop=(kt == 7))
            nc.vector.tensor_scalar(ktd_sb[H:128, 2 * 512:3 * 512],
                                    kv0_ps[0:H, :], bkv_sb[0:H, :], None, add)
            nc.vector.tensor_scalar(kvT_sb[:, 2 * 512:3 * 512], kv0_ps[:],
                                    bkv_sb[:], None, add)
            nc.vector.tensor_scalar(vtd_sb[:, 2 * 512:3 * 512],
                                    kv0_ps[H:128, :], bkv_sb[H:128, :],
                                    None, add)
            pp1.release()

            # slot A AV p2,p3 (exps A p2,p3 done by now)
            for p in range(2):
                for j in range(2):
                    av_accum(avA_e, avA_o, 4 + 2 * p + j, wA2[p][j],
                             False, 4 + 2 * p + j == 7)

            # slot B scores kt 8..11 + V transposes
            for p in (6, 7):
                kt0, kt1 = kts[2 * p], kts[2 * p + 1]
                s0, s1 = score_pair(1, kt0, kt1)
                wB[p] = exp_pair(1, kt0, kt1, s0, s1)
            for pr in (4, 5):
                v_transpose(pr)

            # ---- slot A merge on DVE (ACT is mid-exp); also frees the
            # av pool for slot B
            oavA = ep.tile([H + 1, 512], DT, tag="oavA", name="oavA")
            ocA = ep.tile([H + 1, 512], F32, tag="ocA", name="ocA")
            for j in range(4):
                js = slice(j * 128, (j + 1) * 128)
                nc.vector.tensor_copy(ocA[:, js], avA_e[:, js])
                nc.vector.tensor_tensor(oavA[:, js], ocA[:, js],
                                        avA_o[:, js], add)

            # slot B AV kt 0..7
            avB_e = avp.tile([H + 1, 512], F32, tag="avE", name="avE")
            avB_o = avp.tile([H + 1, 512], F32, tag="avO", name="avO")
            for p in range(4):
                for j in range(2):
                    i = 2 * p + j
                    av_accum(avB_e, avB_o, kts[i], wB[p][j], i == 0, False)

            # slot A transpose + normalize + store
            for j in range(4):
                tr_ps = sp.tile([128, H + 1], DT, tag="score", name="otrA")
                nc.tensor.transpose(tr_ps[:], oavA[:, j * 128:(j + 1) * 128],
                                    id16_sb[:])
                r_sb = ep.tile([128, 1], F32, tag="recip", name="recip")
                nc.vector.reciprocal(r_sb[:], tr_ps[:, H:H + 1])
                o_col = j * H
                nc.vector.tensor_scalar_mul(
                    o_all[:, o_col:o_col + H], tr_ps[:, 0:H], r_sb[:])
            nc.sync.dma_start(out[:, 0:4 * H], o_all[:, 0:4 * H])

            # slot B AV kt 12..15, 8..11
            for p in (4, 5, 6, 7):
                for j in range(2):
                    i = 2 * p + j
                    av_accum(avB_e, avB_o, kts[i], wB[p][j], False, i == 15)

            # ---- slot B tail: merge on ACT (free after exps) + DVE
            oavB = ep.tile([H + 1, 512], DT, tag="oavB", name="oavB")
            ocB = ep.tile([H + 1, 512], F32, tag="ocB", name="ocB")
            for j in range(4):
                js = slice(j * 128, (j + 1) * 128)
                nc.scalar.activation(ocB[:, js], avB_e[:, js], Copy)
                nc.vector.tensor_tensor(oavB[:, js], ocB[:, js],
                                        avB_o[:, js], add)
                tr_ps = sp.tile([128, H + 1], DT, tag="score", name="otrB")
                nc.tensor.transpose(tr_ps[:], oavB[:, js], id16_sb[:])
                r_sb = ep.tile([128, 1], F32, tag="recip", name="recip")
                nc.vector.reciprocal(r_sb[:], tr_ps[:, H:H + 1])
                o_col = (4 + j) * H
                nc.vector.tensor_scalar_mul(
                    o_all[:, o_col:o_col + H], tr_ps[:, 0:H], r_sb[:])
            nc.scalar.dma_start(out[:, 4 * H:8 * H], o_all[:, 4 * H:8 * H])

            for pool in (avp, sp):
                pool.release()

    nc.compile()
    return nc


def _host_inputs(x, Wq, bq, Wk, bk, Wv, bv):
    """Build the 8 per-core input maps (all SBUF-layout, fp16/f32)."""
    f16 = np.float16
    Wkv = np.concatenate([Wk, Wv], axis=1)          # [D, 128]

    cst16_np = np.zeros((128, C16_N), dtype=f16)
    for kt in range(8):
        cst16_np[:, C_WKV + kt * 128:C_WKV + (kt + 1) * 128] = \
            Wkv[kt * 128:(kt + 1) * 128, :]
        cst16_np[:, C_WQ + kt * H:C_WQ + (kt + 1) * H] = \
            Wq[kt * 128:(kt + 1) * 128, :]
    cst16_np[:, C_IDV:C_IDV + H] = np.concatenate(
        [np.eye(H), np.eye(H)], axis=0)
    cst16_np[0:H + 1, C_ID16:C_ID16 + H + 1] = np.eye(H + 1)

    in_maps = []
    for c in range(8):
        b = c // 2
        cA, cB = c % 2, 3 - c % 2
        perm = (cA, 1 - cA, 5 - cB, cB)        # chunk order along k
        xTp = np.concatenate(
            [x[b, p * CH:(p + 1) * CH].T for p in perm], axis=1)  # [D, S]
        xTp = xTp.astype(f16)
        xk_np = np.zeros((128, 16 * 1024), dtype=f16)
        for kt in range(8):
            xk_np[:, kt * 1024:(kt + 1) * 1024] = \
                xTp[kt * 128:(kt + 1) * 128, 0:1024]
            xk_np[:, 8192 + kt * 512:8192 + (kt + 1) * 512] = \
                xTp[kt * 128:(kt + 1) * 128, 1536:2048]
            xk_np[:, 12288 + kt * 512:12288 + (kt + 1) * 512] = \
                xTp[kt * 128:(kt + 1) * 128, 1024:1536]
        # k_global of permuted position p: perm[p//512]*512 + p%512
        pos = np.arange(S)
        kg = np.array(perm)[pos // CH] * CH + pos % CH
        thr_np = np.zeros((128, 2 * NKT), dtype=np.float32)
        p = np.arange(128)
        for slot, ck in enumerate((cA, cB)):
            for kt in range(NKT):
                thr_np[:, slot * NKT + kt] = kg[kt * 128 + p] - ck * CH
        thrb_np = np.zeros((128, 2 * NKT), dtype=np.float32)
        for slot in range(2):
            for kt in range(NKT):
                diag = (slot == 0 and kt < 4) or (slot == 1 and kt >= 12)
                if diag:
                    continue
                col = thr_np[:, slot * NKT + kt]
                if np.all(col <= 0):
                    continue          # fully visible -> bias 0
                thrb_np[:, slot * NKT + kt] = -1e5   # fully masked
        qio = np.arange(CH, dtype=np.float32)[None, :]
        msk_np = np.zeros((128, 8 * 512), dtype=f16)
        for m in range(8):
            idx = m if m < 4 else NKT + 8 + m
            msk_np[:, m * 512:(m + 1) * 512] = \
                (qio >= thr_np[:, idx:idx + 1]).astype(f16)
        cst32_np = np.zeros((128, C32_N), dtype=np.float32)
        cst32_np[:, C_BKV] = np.concatenate([bk, bv])
        cst32_np[0:H, C_BQ] = bq
        cst32_np[:, C_THRB:C_THRB + 2 * NKT] = thrb_np
        in_maps.append({
            "xk": xk_np, "cst16": cst16_np, "cst32": cst32_np,
            "mskd": msk_np,
        })
    return in_maps


def _gather(results, dtype):
    y = np.zeros((B, S, H), dtype=dtype)
    for c in range(8):
        b = c // 2
        cA, cB = c % 2, 3 - c % 2
        o = results[c]["out"]
        for slot, ck in enumerate((cA, cB)):
            for j in range(4):
                col = (slot * 4 + j) * H
                y[b, ck * CH + j * 128:ck * CH + (j + 1) * 128] = \
                    o[:, col:col + H]
    return y


def get_nc():
    if "nc" not in _CACHE:
        _CACHE["nc"] = _build_nc()
    return _CACHE["nc"]


def kernel(x, Wq, bq, Wk, bk, Wv, bv, _trace=False, _trace_kwargs=None):
    from concourse.bass_utils import run_bass_kernel_spmd

    x = np.asarray(x, dtype=np.float32)
    Wq, bq = np.asarray(Wq, np.float32), np.asarray(bq, np.float32)
    Wk, bk = np.asarray(Wk, np.float32), np.asarray(bk, np.float32)
    Wv, bv = np.asarray(Wv, np.float32), np.asarray(bv, np.float32)

    nc = get_nc()
    in_maps = _host_inputs(x, Wq, bq, Wk, bk, Wv, bv)
    res = run_bass_kernel_spmd(
        nc, in_maps, core_ids=list(range(8)),
        trace=_trace, **(_trace_kwargs or {}))
    _CACHE["last_result"] = res
    return _gather(res.results, x.dtype)


# revision 7
# speedup vs baseline: 1.1251x; 1.1251x over previous
"""Trainium2 Bass kernel: single-head causal attention, SPMD over 8 NeuronCores.

Problem: x [4, 2048, 1024] f32; Wq/Wk/Wv [1024, 64]; bq/bk/bv [64].
  q,k,v = x@W + b ; out = softmax(causal(q k^T / 8)) @ v  -> [4, 2048, 64]

Sharding (uniform SPMD structure on every core):
  core c -> batch b = c//2 ; query chunks (cA, cB) = (c%2, 3-c%2), 512 rows
  each (pairing an early with a late chunk balances causal work).  Every core
  computes K/V for its batch's full 2048 rows.

Key layout trick: the k-axis is permuted PER CORE to chunk order
  [cA, 1-cA, 5-cB, cB], so the core's own query columns sit at STATIC
  positions 0:512 and 1536:2048 of the K/V input -- Q projection needs no
  separate input tensor.  Causality comes from host-precomputed per-core 0/1
  mask tiles (diagonal tiles) and exp-bias kills (fully-masked tiles), which
  absorb the permutation.

  Projections produce Q^T/K^T/V^T [64, rows]; scores are computed transposed
  ([k_part, q_free]) so the attention-weight matrix feeds the AV matmul as
  the moving operand; V is re-transposed through 16 small PE transposes; a
  65th "ones" row on the V tiles makes the AV matmul accumulate the softmax
  denominator for free.  Score matmuls (K=64) are row-packed in pairs into
  disjoint PE row-groups; the partition-64:127 K^T/Q^T duplicates they need
  are produced by partition-shifted bias-adds straight from PSUM (DVE/ACT
  read partitions 0:64, write 64:128) -- cheaper than SBUF->SBUF DMA
  (~0.65us sequencer issue + ~2us completion each).

Schedule: the hard resource is the ACT engine (24 exp tiles x ~0.58us,
  1 elem/cycle/lane regardless of dtype).  x streams K-MAJOR -- four
  1MB blocks of k-positions (K0=0:512, K1=512:1024, K3=1536:2048,
  K2=1024:1536), each holding all 8 d-tiles for those positions -- so the
  first K/V block (and the first scores-exp) is ready after ~1MB of
  stream + 16 matmuls instead of after the whole 2.5MB h0 half.  The exp
  chain then runs near-gapless while the remaining blocks stream/project.
  Inputs are FEW LARGE DMAs (one dma_start spans all 16 SDMA engines at
  ~340GB/s; issue costs ~0.65us sequencer time each, so count is key) in
  priority order on the sync queue.  Slot-A attention interleaves with the
  later projections on the PE (PSUM: proj kv 1 + q 2 + score 3 + av 2 = 8
  banks; the 2-bank av pool is reused A->B).  Slot-A merge runs on DVE
  (ACT is mid-exp); slot-B merge on ACT (free after the last exp).

dtypes: fp16 SBUF operands, fp32 PSUM accumulation, fp32 normalize.
"""

import os
import sys

import numpy as np

if "/opt/trn_rl_repo" not in sys.path:
    sys.path.insert(0, "/opt/trn_rl_repo")

B, S, D, H = 4, 2048, 1024, 64
CH = 512          # query chunk width
QR = 2 * CH       # query rows per core
NKT = S // 128    # 16 k-tiles of 128
SCALE = 1.0 / np.sqrt(H)

# cst16 column layout (fp16), one DMA
C_WKV = 0          # [128, 1024] 8 d blocks of [128, 128]
C_WQ = 1024        # [128, 512]  8 d blocks of [128, 64]
C_IDV = 1536       # [128, 64]   eye(64) stacked twice
C_ID16 = 1600      # [65, 65]    eye(65)
C16_N = 1696       # padded total

# cst32 column layout (f32)
C_BKV = 0          # [128, 1]
C_BQ = 1           # [64, 1]
C_THRB = 2         # [128, 32] exp bias: 0 or -1e5 (fully-masked kill)
C32_N = 34

# xk k-major block layout: block bi covers k-positions KPOS[bi]:+512 with
# 8 d-tiles of [128, 512] each; stream order K0, K1, K3, K2.
KPOS = (0, 512, 1536, 1024)
# kvT 512-col block index (nb) per stream block
KNB = (0, 1, 3, 2)

_CACHE = {}


def _build_nc():
    import concourse.bacc as bacc
    import concourse.mybir as mybir
    import concourse.tile as tile

    DT = mybir.dt.float16
    F32 = mybir.dt.float32
    Exp = mybir.ActivationFunctionType.Exp
    Copy = mybir.ActivationFunctionType.Copy
    Ident = mybir.ActivationFunctionType.Identity
    mult = mybir.AluOpType.mult
    add = mybir.AluOpType.add

    nc = bacc.Bacc("TRN2", target_bir_lowering=False, debug=False, num_devices=8)

    xk = nc.dram_tensor("xk", [128, 16 * 1024], DT, kind="ExternalInput")
    cst16 = nc.dram_tensor("cst16", [128, C16_N], DT, kind="ExternalInput")
    cst32 = nc.dram_tensor("cst32", [128, C32_N], F32, kind="ExternalInput")
    mskd = nc.dram_tensor("mskd", [128, 8 * 512], DT, kind="ExternalInput")
    out = nc.dram_tensor("out", [128, 8 * H], DT, kind="ExternalOutput")

    with tile.TileContext(nc) as tc:
        with (
            tc.tile_pool(name="const", bufs=1) as cp,
            tc.tile_pool(name="work", bufs=8) as wp,
            tc.tile_pool(name="epi", bufs=4) as ep,
        ):
            # ---- head: large DMAs in priority order on the sync queue ----
            cst16_sb = cp.tile([128, C16_N], DT, tag="cst16", name="cst16")
            nc.scalar.dma_start(cst16_sb[:], cst16[:])
            xb = []          # SBUF chunk tiles: K0a, K0b, K1, K3, K2
            for ci, (c0, c1) in enumerate(
                    [(0, 2048), (2048, 4096), (4096, 8192),
                     (8192, 12288), (12288, 16384)]):
                t = cp.tile([128, c1 - c0], DT, tag=f"xb{ci}", name=f"xb{ci}")
                xb.append(t)
                nc.sync.dma_start(t[:], xk[:, c0:c1])
                if ci == 1:   # masks after K0, before K1 (needed ~t+18us)
                    msk_sb = cp.tile([128, 8 * 512], DT, tag="msk", name="msk")
                    nc.sync.dma_start(msk_sb[:], mskd[:])
            cst32_sb = cp.tile([128, C32_N], F32, tag="cst32", name="cst32")
            nc.gpsimd.dma_start(cst32_sb[:], cst32[:])

            wkv_sb = cst16_sb[:, C_WKV:C_WKV + 1024]
            wq_sb = cst16_sb[:, C_WQ:C_WQ + 512]
            idv_sb = cst16_sb[:, C_IDV:C_IDV + H]
            id16_sb = cst16_sb[0:H + 1, C_ID16:C_ID16 + H + 1]
            bkv_sb = cst32_sb[:, C_BKV:C_BKV + 1]
            bq_sb = cst32_sb[0:H, C_BQ:C_BQ + 1]
            thrb_sb = cst32_sb[:, C_THRB:C_THRB + 2 * NKT]

            def xs(bi, d):    # stream block bi, d-tile d -> [128, 512]
                if bi == 0:
                    return xb[d // 4][:, (d % 4) * 512:(d % 4 + 1) * 512]
                return xb[bi + 1][:, d * 512:(d + 1) * 512]

            kvT_sb = cp.tile([128, S], DT, tag="kvT", name="kvT")  # 0:64 K^T, 64:128 V^T
            qT_sb = cp.tile([H, QR], DT, tag="qT", name="qT")      # A cols 0:512, B 512:1024
            v_sb = cp.tile([128, NKT * (H + 1)], DT, tag="v", name="v")
            # duplicates at partitions 64:127 for row-packed score pairs
            ktd_sb = cp.tile([128, S], DT, tag="ktd", name="ktd")
            qTd_sb = cp.tile([128, QR], DT, tag="qTd", name="qTd")
            vtd_sb = cp.tile([64, S], DT, tag="vtd", name="vtd")
            o_all = cp.tile([128, 8 * H], DT, tag="oall", name="oall")

            nc.vector.memset(v_sb[:], 1.0)

            pk = tc.alloc_tile_pool(name="proj_ps", bufs=1, space="PSUM")
            pq = tc.alloc_tile_pool(name="q_ps", bufs=2, space="PSUM")
            sp = tc.alloc_tile_pool(name="score_ps", bufs=3, space="PSUM")
            avp = tc.alloc_tile_pool(name="av_ps", bufs=1, space="PSUM")

            def kv_block(bi):
                """8 accumulating matmuls for one 512-position k block, then
                DVE epilogue (ktd dup first: it unblocks row-packed scores)."""
                kv_ps = pk.tile([128, 512], F32, tag="kvps", name="kvps")
                for d in range(8):
                    nc.tensor.matmul(
                        kv_ps[:], wkv_sb[:, d * 128:(d + 1) * 128],
                        xs(bi, d), start=(d == 0), stop=(d == 7))
                nb = KNB[bi]
                cs = slice(nb * 512, (nb + 1) * 512)
                nc.vector.tensor_scalar(ktd_sb[H:128, cs], kv_ps[0:H, :],
                                        bkv_sb[0:H, :], None, add)
                nc.vector.tensor_scalar(kvT_sb[:, cs], kv_ps[:],
                                        bkv_sb[:], None, add)
                nc.vector.tensor_scalar(vtd_sb[:, cs], kv_ps[H:128, :],
                                        bkv_sb[H:128, :], None, add)

            def q_block(bi, slot):
                q_ps = pq.tile([H, 512], F32, tag="qps", name="qps")
                for d in range(8):
                    nc.tensor.matmul(
                        q_ps[:], wq_sb[:, d * H:(d + 1) * H],
                        xs(bi, d), start=(d == 0), stop=(d == 7))
                cs = slice(slot * 512, (slot + 1) * 512)
                if slot == 0:     # ACT is idle pre-exp; keep DVE for kv epis
                    nc.scalar.activation(qT_sb[:, cs], q_ps[:], Ident,
                                         bias=bq_sb[:])
                    nc.scalar.activation(qTd_sb[H:128, cs], q_ps[:], Ident,
                                         bias=bq_sb[:])
                else:             # ACT is mid-exp by now
                    nc.vector.tensor_scalar(qT_sb[:, cs], q_ps[:],
                                            bq_sb[:], None, add)
                    nc.vector.tensor_scalar(qTd_sb[H:128, cs], q_ps[:],
                                            bq_sb[:], None, add)

            def v_transpose(pr):
                k0, k1 = 2 * pr, 2 * pr + 1
                t0 = sp.tile([128, H], DT, tag="score", name="vtr0")
                nc.tensor.transpose(
                    t0[:], vtd_sb[:, k0 * 128:(k0 + 1) * 128],
                    idv_sb[0:H, :], tile_position=(0, 0))
                t1 = sp.tile([128, H], DT, tag="score", name="vtr1")
                nc.tensor.transpose(
                    t1[:], kvT_sb[64:128, k1 * 128:(k1 + 1) * 128],
                    idv_sb[64:64 + H, :], tile_position=(64, 0))
                nc.vector.tensor_copy(
                    v_sb[:, k0 * (H + 1):k0 * (H + 1) + H], t0[:])
                nc.vector.tensor_copy(
                    v_sb[:, k1 * (H + 1):k1 * (H + 1) + H], t1[:])

            def score_pair(slot, kt0, kt1):
                s0 = sp.tile([128, 512], F32, tag="score", name="score0")
                nc.tensor.matmul(
                    s0[:], kvT_sb[0:H, kt0 * 128:(kt0 + 1) * 128],
                    qT_sb[:, slot * 512:(slot + 1) * 512],
                    start=True, stop=True, tile_position=(0, 0))
                s1 = sp.tile([128, 512], F32, tag="score", name="score1")
                nc.tensor.matmul(
                    s1[:], ktd_sb[H:128, kt1 * 128:(kt1 + 1) * 128],
                    qTd_sb[H:128, slot * 512:(slot + 1) * 512],
                    start=True, stop=True, tile_position=(64, 0))
                return s0, s1

            def exp_pair(slot, kt0, kt1, s0, s1):
                res = []
                for kt, s_ps in zip((kt0, kt1), (s0, s1)):
                    idx = slot * NKT + kt
                    w_sb = wp.tile([128, 512], DT, tag="wexp", name="wexp")
                    nc.scalar.activation(w_sb[:], s_ps[:], Exp,
                                         bias=thrb_sb[:, idx:idx + 1],
                                         scale=float(SCALE))
                    diag = (slot == 0 and kt < 4) or (slot == 1 and kt >= 12)
                    if diag:
                        m = kt if slot == 0 else kt - 8
                        wm_sb = wp.tile([128, 512], DT, tag="wm", name="wm")
                        nc.vector.tensor_tensor(
                            wm_sb[:], w_sb[:],
                            msk_sb[:, m * 512:(m + 1) * 512], mult)
                        res.append(wm_sb)
                    else:
                        res.append(w_sb)
                return res

            def av_accum(av_e, av_o, kt, w_av, first, last):
                vs = slice(kt * (H + 1), (kt + 1) * (H + 1))
                nc.tensor.matmul(
                    av_e[:], v_sb[0:H, vs], w_av[0:H, :],
                    start=first, stop=last, tile_position=(0, 0))
                nc.tensor.matmul(
                    av_o[:], v_sb[H:128, vs], w_av[H:128, :],
                    start=first, stop=last, tile_position=(64, 0))

            # ================= emission (PE-queue order) =================
            kv_block(0)                       # K0 -> kvT 0:512
            q_block(0, 0)                     # qA (+ ACT idents)

            avA_e = avp.tile([H + 1, 512], F32, tag="avE", name="avE")
            avA_o = avp.tile([H + 1, 512], F32, tag="avO", name="avO")
            sA = [score_pair(0, 2 * p, 2 * p + 1) for p in range(2)]
            wA = [exp_pair(0, 2 * p, 2 * p + 1, *sA[p]) for p in range(2)]
            for pr in range(2):
                v_transpose(pr)

            kv_block(1)                       # K1 -> kvT 512:1024
            sA2 = [score_pair(0, 4 + 2 * p, 5 + 2 * p) for p in range(2)]
            wA2 = [exp_pair(0, 4 + 2 * p, 5 + 2 * p, *sA2[p]) for p in range(2)]
            for pr in range(2, 4):
                v_transpose(pr)

            q_block(2, 1)                     # qB from K3 block (DVE epi)
            kts = list(range(8)) + [12, 13, 14, 15, 8, 9, 10, 11]
            wB = {}
            for p in range(4):                # slot B scores kt 0..7
                kt0, kt1 = kts[2 * p], kts[2 * p + 1]
                s0, s1 = score_pair(1, kt0, kt1)
                wB[p] = exp_pair(1, kt0, kt1, s0, s1)

            for p in range(2):                # slot A AV kt 0..3
                for j in range(2):
                    av_accum(avA_e, avA_o, 2 * p + j, wA[p][j],
                             2 * p + j == 0, False)

            kv_block(2)                       # K3 -> kvT 1536:2048
            for pr in (6, 7):
                v_transpose(pr)
            for p in (4, 5):                  # slot B scores kt 12..15
                kt0, kt1 = kts[2 * p], kts[2 * p + 1]
                s0, s1 = score_pair(1, kt0, kt1)
                wB[p] = exp_pair(1, kt0, kt1, s0, s1)

            for p in range(2):                # slot A AV kt 4..7
                for j in range(2):
                    av_accum(avA_e, avA_o, 4 + 2 * p + j, wA2[p][j],
                             False, 4 + 2 * p + j == 7)

            kv_block(3)                       # K2 -> kvT 1024:1536
            for p in (6, 7):                  # slot B scores kt 8..11
                kt0, kt1 = kts[2 * p], kts[2 * p + 1]
                s0, s1 = score_pair(1, kt0, kt1)
                wB[p] = exp_pair(1, kt0, kt1, s0, s1)
            for pr in (4, 5):
                v_transpose(pr)

            # slot A merge on DVE (ACT mid-exp); frees av pool for slot B
            oavA = ep.tile([H + 1, 512], DT, tag="oavA", name="oavA")
            ocA = ep.tile([H + 1, 512], F32, tag="ocA", name="ocA")
            for j in range(4):
                js = slice(j * 128, (j + 1) * 128)
                nc.vector.tensor_copy(ocA[:, js], avA_e[:, js])
                nc.vector.tensor_tensor(oavA[:, js], ocA[:, js],
                                        avA_o[:, js], add)

            avB_e = avp.tile([H + 1, 512], F32, tag="avE", name="avE")
            avB_o = avp.tile([H + 1, 512], F32, tag="avO", name="avO")
            for p in range(4):                # slot B AV kt 0..7
                for j in range(2):
                    i = 2 * p + j
                    av_accum(avB_e, avB_o, kts[i], wB[p][j], i == 0, False)

            # slot A transpose + normalize + store
            for j in range(4):
                tr_ps = sp.tile([128, H + 1], DT, tag="score", name="otrA")
                nc.tensor.transpose(tr_ps[:], oavA[:, j * 128:(j + 1) * 128],
                                    id16_sb[:])
                r_sb = ep.tile([128, 1], F32, tag="recip", name="recip")
                nc.vector.reciprocal(r_sb[:], tr_ps[:, H:H + 1])
                o_col = j * H
                nc.vector.tensor_scalar_mul(
                    o_all[:, o_col:o_col + H], tr_ps[:, 0:H], r_sb[:])
            nc.sync.dma_start(out[:, 0:4 * H], o_all[:, 0:4 * H])

            for p in (4, 5, 6, 7):            # slot B AV kt 12..15, 8..11
                for j in range(2):
                    i = 2 * p + j
                    av_accum(avB_e, avB_o, kts[i], wB[p][j], False, i == 15)

            # slot B tail: merge on ACT (free after exps) + DVE
            oavB = ep.tile([H + 1, 512], DT, tag="oavB", name="oavB")
            ocB = ep.tile([H + 1, 512], F32, tag="ocB", name="ocB")
            for j in range(4):
                js = slice(j * 128, (j + 1) * 128)
                nc.scalar.activation(ocB[:, js], avB_e[:, js], Copy)
                nc.vector.tensor_tensor(oavB[:, js], ocB[:, js],
                                        avB_o[:, js], add)
                tr_ps = sp.tile([128, H + 1], DT, tag="score", name="otrB")
                nc.tensor.transpose(tr_ps[:], oavB[:, js], id16_sb[:])
                r_sb = ep.tile([128, 1], F32, tag="recip", name="recip")
                nc.vector.reciprocal(r_sb[:], tr_ps[:, H:H + 1])
                o_col = (4 + j) * H
                nc.vector.tensor_scalar_mul(
                    o_all[:, o_col:o_col + H], tr_ps[:, 0:H], r_sb[:])
            nc.scalar.dma_start(out[:, 4 * H:8 * H], o_all[:, 4 * H:8 * H])

            for pool in (avp, sp, pq, pk):
                pool.release()

    nc.compile()
    return nc


def _host_inputs(x, Wq, bq, Wk, bk, Wv, bv):
    """Build the 8 per-core input maps (all SBUF-layout, fp16/f32)."""
    f16 = np.float16
    Wkv = np.concatenate([Wk, Wv], axis=1)          # [D, 128]

    cst16_np = np.zeros((128, C16_N), dtype=f16)
    for d in range(8):
        cst16_np[:, C_WKV + d * 128:C_WKV + (d + 1) * 128] = \
            Wkv[d * 128:(d + 1) * 128, :]
        cst16_np[:, C_WQ + d * H:C_WQ + (d + 1) * H] = \
            Wq[d * 128:(d + 1) * 128, :]
    cst16_np[:, C_IDV:C_IDV + H] = np.concatenate(
        [np.eye(H), np.eye(H)], axis=0)
    cst16_np[0:H + 1, C_ID16:C_ID16 + H + 1] = np.eye(H + 1)

    in_maps = []
    for c in range(8):
        b = c // 2
        cA, cB = c % 2, 3 - c % 2
        perm = (cA, 1 - cA, 5 - cB, cB)        # chunk order along k
        xTp = np.concatenate(
            [x[b, p * CH:(p + 1) * CH].T for p in perm], axis=1)  # [D, S]
        xTp = xTp.astype(f16)
        xk_np = np.zeros((128, 16 * 1024), dtype=f16)
        for bi in range(4):
            kp = KPOS[bi]
            for d in range(8):
                xk_np[:, bi * 4096 + d * 512:bi * 4096 + (d + 1) * 512] = \
                    xTp[d * 128:(d + 1) * 128, kp:kp + 512]
        # k_global of permuted position p: perm[p//512]*512 + p%512
        pos = np.arange(S)
        kg = np.array(perm)[pos // CH] * CH + pos % CH
        thr_np = np.zeros((128, 2 * NKT), dtype=np.float32)
        p = np.arange(128)
        for slot, ck in enumerate((cA, cB)):
            for kt in range(NKT):
                thr_np[:, slot * NKT + kt] = kg[kt * 128 + p] - ck * CH
        thrb_np = np.zeros((128, 2 * NKT), dtype=np.float32)
        for slot in range(2):
            for kt in range(NKT):
                diag = (slot == 0 and kt < 4) or (slot == 1 and kt >= 12)
                if diag:
                    continue
                col = thr_np[:, slot * NKT + kt]
                if np.all(col <= 0):
                    continue          # fully visible -> bias 0
                thrb_np[:, slot * NKT + kt] = -1e5   # fully masked
        qio = np.arange(CH, dtype=np.float32)[None, :]
        msk_np = np.zeros((128, 8 * 512), dtype=f16)
        for m in range(8):
            idx = m if m < 4 else NKT + 8 + m
            msk_np[:, m * 512:(m + 1) * 512] = \
                (qio >= thr_np[:, idx:idx + 1]).astype(f16)
        cst32_np = np.zeros((128, C32_N), dtype=np.float32)
        cst32_np[:, C_BKV] = np.concatenate([bk, bv])
        cst32_np[0:H, C_BQ] = bq
        cst32_np[:, C_THRB:C_THRB + 2 * NKT] = thrb_np
        in_maps.append({
            "xk": xk_np, "cst16": cst16_np, "cst32": cst32_np,
            "mskd": msk_np,
        })
    return in_maps


def _gather(results, dtype):
    y = np.zeros((B, S, H), dtype=dtype)
    for c in range(8):
        b = c // 2
        cA, cB = c % 2, 3 - c % 2
        o = results[c]["out"]
        for slot, ck in enumerate((cA, cB)):
            for j in range(4):
                col = (slot * 4 + j) * H
                y[b, ck * CH + j * 128:ck * CH + (j + 1) * 128] = \
                    o[:, col:col + H]
    return y


def get_nc():
    if "nc" not in _CACHE:
        _CACHE["nc"] = _build_nc()
    return _CACHE["nc"]


def kernel(x, Wq, bq, Wk, bk, Wv, bv, _trace=False, _trace_kwargs=None):
    from concourse.bass_utils import run_bass_kernel_spmd

    x = np.asarray(x, dtype=np.float32)
    Wq, bq = np.asarray(Wq, np.float32), np.asarray(bq, np.float32)
    Wk, bk = np.asarray(Wk, np.float32), np.asarray(bk, np.float32)
    Wv, bv = np.asarray(Wv, np.float32), np.asarray(bv, np.float32)

    nc = get_nc()
    in_maps = _host_inputs(x, Wq, bq, Wk, bk, Wv, bv)
    res = run_bass_kernel_spmd(
        nc, in_maps, core_ids=list(range(8)),
        trace=_trace, **(_trace_kwargs or {}))
    _CACHE["last_result"] = res
    return _gather(res.results, x.dtype)
